# revision 17
# baseline (speedup 1.0000x reference)
import os
import sys

sys.path.insert(0, "/opt/trn_rl_repo")

import numpy as np

import concourse.bass as bass
import concourse.mybir as mybir
import concourse.tile as tile
from concourse import bacc, library_config

# ---------------- problem constants (hardcoded per spec) ----------------
N_NODES = 100000
N_EDGES = 640000
C = 128           # channels (in == out)
P = 128           # partitions
N_CORES = 8
NPC = N_NODES // N_CORES          # 12500 nodes per core
NBLK = (NPC + P - 1) // P         # 98 blocks per core
NPC_PAD = NBLK * P                # 12544
NBANK = 4
BANK = N_NODES // NBANK           # 25000 rows per gather bank (int16-safe)
BPG = 8                           # blocks per gather group

F32 = mybir.dt.float32
BF16 = mybir.dt.bfloat16
I32 = mybir.dt.int32
I16 = mybir.dt.int16


class Layout:
    """Static chunk layout shared by both programs and the host.

    Chunks are ordered call-major: for each group g of up to BPG blocks,
    for each bank k, for each block b in the group, cnpb chunks of 128
    edge slots. All edges of chunk q target block block_of[q] and have
    source col in bank k_of[q].
    """

    def __init__(self, cnpb):
        self.cnpb = cnpb
        self.groups = []
        b = 0
        while b < NBLK:
            gb = min(BPG, NBLK - b)
            self.groups.append((b, gb))
            b += gb
        self.calls = []      # (g, k, chunk_base, ntok)
        self.chunk_block = []
        self.chunk_bank = []
        # chunk index for (block, bank, j)
        self.cell_chunk = np.zeros((NBLK, NBANK, cnpb), np.int64)
        q = 0
        for g, (b0, gb) in enumerate(self.groups):
            for k in range(NBANK):
                self.calls.append((g, k, q, gb * cnpb * P))
                for bi in range(gb):
                    for j in range(cnpb):
                        self.cell_chunk[b0 + bi, k, j] = q
                        self.chunk_block.append(b0 + bi)
                        self.chunk_bank.append(k)
                        q += 1
        self.ech = q


def _build_deg_program(dch):
    """Launch 1: deg = segment_sum(vals, row); D = 1/sqrt(deg+1).
    vals2[m, b*dch + t] holds the t-th in-edge value of the row at
    (block b, lane m); deg is a plain innermost-axis reduction.
    Output d[P, NBLK] with d[m, b] = D of row (block b, lane m)."""
    nc = bacc.Bacc("TRN2", target_bir_lowering=False, debug=False,
                   num_devices=N_CORES)
    vals2_d = nc.dram_tensor("vals2", [P, NBLK * dch], F32,
                             kind="ExternalInput")
    d_d = nc.dram_tensor("d", [P, NBLK], F32, kind="ExternalOutput")

    with tile.TileContext(nc) as tc:
        with tc.tile_pool(name="consts", bufs=1) as consts:
            vals2_t = consts.tile([P, NBLK * dch], F32)
            nc.sync.dma_start(out=vals2_t[:], in_=vals2_d[:])
            deg_sb = consts.tile([P, NBLK], F32)
            nc.vector.tensor_reduce(
                out=deg_sb[:],
                in_=vals2_t[:].rearrange("p (b t) -> p b t", t=dch),
                axis=mybir.AxisListType.X,
                op=mybir.AluOpType.add,
            )
            s_sb = consts.tile([P, NBLK], F32)
            nc.scalar.activation(
                out=s_sb[:], in_=deg_sb[:],
                func=mybir.ActivationFunctionType.Sqrt, bias=1.0,
            )
            d_sb = consts.tile([P, NBLK], F32)
            nc.vector.reciprocal(out=d_sb[:], in_=s_sb[:])
            nc.sync.dma_start(out=d_d[:], in_=d_sb[:])
    nc.compile()
    return nc


def _build_agg_program(layout):
    """Launch 2: dma_gather X[col] per (group, bank), aggregate w*X[col]
    per block via one-hot matmuls, then @W, D-scale, +bias."""
    ech, cnpb = layout.ech, layout.cnpb
    nidx16 = ech * P // 16

    nc = bacc.Bacc("TRN2", target_bir_lowering=False, debug=False,
                   num_devices=N_CORES)
    x_d = nc.dram_tensor("x", [N_NODES, C], BF16, kind="ExternalInput")
    idx_d = nc.dram_tensor("idx16", [P, nidx16], I16, kind="ExternalInput")
    rowloc_d = nc.dram_tensor("rowloc", [P, ech], F32, kind="ExternalInput")
    vals_d = nc.dram_tensor("vals", [P, ech], F32, kind="ExternalInput")
    dcolv_d = nc.dram_tensor("dcolv", [P, ech], F32, kind="ExternalInput")
    down_d = nc.dram_tensor("down", [P, NBLK], F32, kind="ExternalInput")
    iota_d = nc.dram_tensor("iota", [P, P], BF16, kind="ExternalInput")
    wmat_d = nc.dram_tensor("wmat", [C, C], F32, kind="ExternalInput")
    biasb_d = nc.dram_tensor("biasb", [P, C], F32, kind="ExternalInput")
    y_d = nc.dram_tensor("y", [NPC_PAD, C], F32, kind="ExternalOutput")

    with tile.TileContext(nc) as tc:
        with tc.tile_pool(name="consts", bufs=1) as consts, \
             tc.tile_pool(name="meta", bufs=1) as meta, \
             tc.tile_pool(name="oh", bufs=6) as oh_pool, \
             tc.tile_pool(name="xg", bufs=8) as xg_pool, \
             tc.tile_pool(name="blk", bufs=8) as blk_pool, \
             tc.tile_pool(name="pmt", bufs=2, space="PSUM") as pmt_pool, \
             tc.tile_pool(name="pout", bufs=2, space="PSUM") as pout_pool:

            nc.gpsimd.load_library(library_config.mlp)

            iota_t = consts.tile([P, P], BF16)
            nc.sync.dma_start(out=iota_t[:], in_=iota_d[:])
            wmat_t = consts.tile([C, C], F32)
            nc.sync.dma_start(out=wmat_t[:], in_=wmat_d[:])
            biasb_t = consts.tile([P, C], F32)
            nc.sync.dma_start(out=biasb_t[:], in_=biasb_d[:])
            down_t = consts.tile([P, NBLK], F32)
            nc.sync.dma_start(out=down_t[:], in_=down_d[:])

            idx_t = meta.tile([P, nidx16], I16)
            nc.sync.dma_start(out=idx_t[:], in_=idx_d[:])
            rowloc_t = meta.tile([P, ech], F32)
            nc.sync.dma_start(out=rowloc_t[:], in_=rowloc_d[:])
            vals_t = meta.tile([P, ech], F32)
            nc.sync.dma_start(out=vals_t[:], in_=vals_d[:])
            dcolv_t = meta.tile([P, ech], F32)
            nc.sync.dma_start(out=dcolv_t[:], in_=dcolv_d[:])
            w_sb = meta.tile([P, ech], F32)
            nc.vector.tensor_tensor(out=w_sb[:], in0=vals_t[:],
                                    in1=dcolv_t[:], op=mybir.AluOpType.mult)

            for g, (b0, gb) in enumerate(layout.groups):
                stage = []
                for k in range(NBANK):
                    _, _, qbase, ntok = layout.calls[g * NBANK + k]
                    xg = xg_pool.tile([P, BPG * cnpb * C], BF16,
                                      name=f"xg{k}", tag="xg")
                    nc.gpsimd.dma_gather(
                        out_ap=xg[:, :gb * cnpb * C].rearrange(
                            "p (s c) -> p s c", c=C),
                        in_ap=x_d[k * BANK:(k + 1) * BANK, :],
                        idxs_ap=idx_t[:, qbase * 8:qbase * 8 + ntok // 16],
                        num_idxs=ntok, num_idxs_reg=ntok,
                        elem_size=C, single_packet=False,
                    )
                    stage.append(xg)
                for bi in range(gb):
                    b = b0 + bi
                    mt = pmt_pool.tile([C, P], F32, name="mt")
                    nch = NBANK * cnpb
                    for i in range(nch):
                        k, j = i // cnpb, i % cnpb
                        ch = int(layout.cell_chunk[b, k, j])
                        oh = oh_pool.tile([P, P], BF16, name="oh2")
                        nc.vector.tensor_scalar(
                            out=oh[:], in0=iota_t[:],
                            scalar1=rowloc_t[:, ch:ch + 1],
                            scalar2=w_sb[:, ch:ch + 1],
                            op0=mybir.AluOpType.is_equal,
                            op1=mybir.AluOpType.mult,
                        )
                        s = bi * cnpb + j
                        nc.tensor.matmul(
                            out=mt[:], lhsT=stage[k][:, s * C:(s + 1) * C],
                            rhs=oh[:],
                            start=(i == 0), stop=(i == nch - 1),
                        )
                    mts = blk_pool.tile([C, P], F32, name="mts")
                    nc.vector.tensor_copy(out=mts[:], in_=mt[:])
                    outp = pout_pool.tile([P, C], F32, name="outp")
                    nc.tensor.matmul(out=outp[:], lhsT=mts[:], rhs=wmat_t[:],
                                     start=True, stop=True)
                    ys = blk_pool.tile([P, C], F32, name="ys")
                    nc.scalar.activation(
                        out=ys[:], in_=outp[:],
                        func=mybir.ActivationFunctionType.Copy,
                        scale=down_t[:, b:b + 1],
                    )
                    nc.vector.tensor_tensor(out=ys[:], in0=ys[:],
                                            in1=biasb_t[:],
                                            op=mybir.AluOpType.add)
                    nc.sync.dma_start(out=y_d[b * P:(b + 1) * P, :], in_=ys[:])
    nc.compile()
    return nc


def _assign_rows_banked(cnt4):
    """Greedily assign NPC rows to NBLK blocks (<=128 rows each), balancing
    per-bank edge loads. cnt4: [NPC, NBANK] per-row in-edge counts by source
    bank. Returns (block[NPC], lane[NPC], load[NBLK, NBANK])."""
    npc = cnt4.shape[0]
    order = np.argsort(-cnt4.sum(1), kind="stable")
    load = np.zeros((NBLK, NBANK), np.int64)
    count = np.zeros(NBLK, np.int64)
    block = np.empty(npc, np.int32)
    lane = np.empty(npc, np.int32)
    for r in order:
        cand = count < P
        # resulting max per-bank load if row r joins each candidate block
        res = (load + cnt4[r][None, :]).max(1)
        res[~cand] = 1 << 60
        b = int(np.argmin(res))
        block[r] = b
        lane[r] = count[b]
        count[b] += 1
        load[b] += cnt4[r]
    return block, lane, load


def _preprocess(row, col, vals):
    row = np.asarray(row).astype(np.int64)
    col = np.asarray(col).astype(np.int64)
    vals = np.asarray(vals).astype(np.float32)

    owner = row // NPC
    ebank = col // BANK
    node_block = np.empty(N_NODES, np.int32)
    node_lane = np.empty(N_NODES, np.int32)

    core_edges = []
    cnpb = 1
    for c in range(N_CORES):
        m = owner == c
        r_loc = (row[m] - c * NPC).astype(np.int64)
        cnt4 = np.zeros((NPC, NBANK), np.int64)
        np.add.at(cnt4, (r_loc, ebank[m]), 1)
        blk, lane, load = _assign_rows_banked(cnt4)
        node_block[c * NPC:(c + 1) * NPC] = blk
        node_lane[c * NPC:(c + 1) * NPC] = lane
        cnpb = max(cnpb, (int(load.max()) + P - 1) // P)
        core_edges.append((r_loc, col[m], vals[m], blk))

    # per-row in-degree chunk count for the reduce-based deg kernel
    all_indeg = np.bincount(row, minlength=N_NODES)
    dch = max(1, int(all_indeg.max()))

    layout = Layout(cnpb)
    ech = layout.ech
    idx16 = np.zeros((N_CORES, P, ech * P // 16), np.int16)
    rowloc = np.full((N_CORES, ech, P), -1.0, np.float32)
    valsa = np.zeros((N_CORES, ech, P), np.float32)
    ecols = np.zeros((N_CORES, ech, P), np.int64)
    vals2 = np.zeros((N_CORES, P, NBLK * dch), np.float32)

    for c in range(N_CORES):
        r_loc, e_val_c = core_edges[c][0], core_edges[c][2]
        order_r = np.argsort(r_loc, kind="stable")
        r_s = r_loc[order_r]
        starts_r = np.searchsorted(r_s, np.arange(NPC))
        offs_r = np.arange(len(r_s)) - starts_r[r_s]
        lane_arr = node_lane[c * NPC:(c + 1) * NPC].astype(np.int64)
        blk_arr = node_block[c * NPC:(c + 1) * NPC].astype(np.int64)
        vals2[c, lane_arr[r_s], blk_arr[r_s] * dch + offs_r] = \
            e_val_c[order_r]

    for c in range(N_CORES):
        r_loc, e_col, e_val, blk = core_edges[c]
        eb = blk[r_loc].astype(np.int64)
        ek = e_col // BANK
        cell = eb * NBANK + ek          # cell id: block-major, bank minor
        order = np.argsort(cell, kind="stable")
        cell_s = cell[order]
        starts = np.searchsorted(cell_s, np.arange(NBLK * NBANK))
        offs = np.arange(len(cell_s)) - starts[cell_s]
        chunk = layout.cell_chunk[eb[order], ek[order], offs // P]
        lane = offs % P
        rowloc[c, chunk, lane] = node_lane[c * NPC:(c + 1) * NPC][
            r_loc[order]]
        valsa[c, chunk, lane] = e_val[order]
        ecols[c, chunk, lane] = e_col[order]
        # int16 gather tokens: token t of chunk q is slot (q*128 + lane);
        # index value relative to its bank base.
        tok = np.zeros(ech * P, np.int16)
        tok[chunk * P + lane] = (e_col[order] - ek[order] * BANK).astype(
            np.int16)
        wrap = tok.reshape(ech * P // 16, 16).T          # [16, ntok/16]
        idx16[c] = np.tile(wrap, (8, 1))

    rowloc = np.ascontiguousarray(rowloc.transpose(0, 2, 1))
    valsa = np.ascontiguousarray(valsa.transpose(0, 2, 1))
    ecols = np.ascontiguousarray(ecols.transpose(0, 2, 1))
    return (layout, dch, idx16, rowloc, valsa, vals2, ecols,
            node_block, node_lane)


_CACHE = {}


def _get_programs(layout, dch):
    key = (layout.ech, layout.cnpb, dch)
    if key not in _CACHE:
        _CACHE[key] = (_build_deg_program(dch),
                       _build_agg_program(layout))
    return _CACHE[key]


def _run(nc, in_maps):
    if os.environ.get("KERNEL_SIM"):
        from concourse import bass_interp
        sim = bass_interp.MultiCoreSim(nc, N_CORES)
        for c in range(N_CORES):
            for k, v in in_maps[c].items():
                sim.cores[c].tensor(k)[:] = v
        sim.simulate()
        out_names = [
            a.memorylocations[0].name
            for a in nc.m.functions[0].allocations
            if isinstance(a, mybir.MemoryLocationSet)
            and a.kind == "ExternalOutput"
        ]
        return [{n: np.array(sim.cores[c].mem_tensor(n)) for n in out_names}
                for c in range(N_CORES)]
    from concourse.bass_utils import run_bass_kernel_spmd
    res = run_bass_kernel_spmd(nc, in_maps, core_ids=list(range(N_CORES)))
    return res.results


def kernel(row, col, vals, X, weights, bias):
    X = np.asarray(X).astype(np.float32)
    weights = np.asarray(weights).astype(np.float32)
    bias = np.asarray(bias).astype(np.float32)

    (layout, dch, idx16, rowloc, valsa, vals2, ecols,
     node_block, node_lane) = _preprocess(row, col, vals)
    nc_deg, nc_agg = _get_programs(layout, dch)

    import ml_dtypes
    iota = np.ascontiguousarray(
        np.broadcast_to(np.arange(P, dtype=np.float32), (P, P))).astype(
        ml_dtypes.bfloat16)
    biasb = np.ascontiguousarray(np.broadcast_to(bias, (P, C)))

    # ---- launch 1: degree/D ----
    deg_maps = [{"vals2": vals2[c]} for c in range(N_CORES)]
    deg_res = _run(nc_deg, deg_maps)
    d_all = np.stack([deg_res[c]["d"] for c in range(N_CORES)])  # [8, P, NBLK]

    # host routing of D to edge slots (pure indexing)
    c_of = ecols // NPC
    dcolv = d_all[c_of, node_lane[ecols].astype(np.int64),
                  node_block[ecols].astype(np.int64)].astype(np.float32)

    down = np.ascontiguousarray(d_all)  # already [P, NBLK] per core

    # ---- launch 2: gather + aggregate ----
    x_bf16 = X.astype(ml_dtypes.bfloat16)
    agg_maps = []
    for c in range(N_CORES):
        agg_maps.append({
            "x": x_bf16, "idx16": idx16[c], "rowloc": rowloc[c],
            "vals": valsa[c], "dcolv": dcolv[c], "down": down[c],
            "iota": iota, "wmat": weights, "biasb": biasb,
        })
    agg_res = _run(nc_agg, agg_maps)
    ys = [agg_res[c]["y"] for c in range(N_CORES)]

    out = np.empty((N_NODES, C), np.float32)
    for c in range(N_CORES):
        nb = node_block[c * NPC:(c + 1) * NPC].astype(np.int64)
        nl = node_lane[c * NPC:(c + 1) * NPC].astype(np.int64)
        out[c * NPC:(c + 1) * NPC] = ys[c][nb * P + nl]
    return out
